# revision 5
# baseline (speedup 1.0000x reference)
"""Trainium2 Bass kernel for the blob-layer problem.

Computes out[b, c] = sum_hw x[b, hw] * curves[hw, c] / (H*W) where
curves[hw, c] = clip(factor_c * exp(-((xs-px_c)^2 + (ys-py_c)^2)/s2_c) * w_c).

Strategy (8 NeuronCores, SPMD):
- The Gaussian is SEPARABLE: exp(-((x-px)^2+(y-py)^2)/s2) =
  Ex[w,c] * Ey[h,c]. Host computes the tiny 1-D tables; the device
  builds each tile's exp field with DVE multiplies of partition-
  replicated table tiles — no G matmul, no ACT Exp. The tensor engine
  does ONLY the main contraction: 2 fp16 matmuls per 128-pixel tile
  (one per 128-batch chunk) accumulating into PSUM.
- 2D core grid (4 y-bands x 2 x-halves), block = 56x112 pixels per
  core, tiled 7x7 as (8 rows x 16 cols) 128-pixel tiles. 2D pruning
  keeps columns with corner distance^2/s2 <= T=9, capped at NC=320
  (measured pruning-only rel err 1.7e-3 vs the 2e-2 gate).
- TRN2 ramps the PE clock 0.65 -> 1.2 -> 2.4 GHz with ~3us of
  continuous execution, so the schedule keeps the PE saturated:
  x streams per-tile on the sync HW-DGE queue (512B/partition
  descriptors, delivery 182ns/tile < 266ns/tile consumption),
  tables go compact over the scalar queue and are replicated by
  stride-0 broadcast DMAs, junk warm-up matmuls start the clock
  ramp before real data lands, and e is produced a full 7-tile row
  per DVE op (free-dim broadcast of Ey against the whole Ex row).
- factor*w/npix, the column gather/unpad, and the cross-core sum are
  applied on the host; the clip never binds when max|factor*w| <=
  CAP (exp <= 1), which is asserted.
"""
import os
import sys

sys.path.insert(0, "/opt/trn_rl_repo")

import numpy as np

import concourse.bass as bass
import concourse.bacc as bacc
import concourse.tile as tile
from concourse import mybir
from concourse.bass_utils import run_bass_kernel_spmd

H, W, B, C = 224, 224, 256, 1024
NDEV = 8
GY, GX = 4, 2             # core grid: 4 y-bands x 2 x-halves
BY, BX = H // GY, W // GX  # 56 x 112 block per core
TY, TX = 8, 16            # tile = 8 rows x 16 cols = 128 pixels
NI, NJ = BY // TY, BX // TX  # 7 x 7 tiles
NT = NI * NJ              # 49 tiles
NC = 320                  # kept/padded columns per core
EPS = 0.001
CAP = 2000.0
NPIX = float(H * W)
T_PRUNE = 9.0
WARMUP = 4                # junk matmuls that start the PE clock ramp

last_results = None       # BassKernelResults of the most recent run (for profiling)


def _build_program():
    nc = bacc.Bacc()
    f32 = mybir.dt.float32
    f16 = mybir.dt.float16

    d_xT = nc.declare_dram_parameter("xT", [NT, 128, B], f16, isOutput=False)
    d_Ey = nc.declare_dram_parameter("Ey", [BY, NC], f16, isOutput=False)
    d_Ex = nc.declare_dram_parameter("Ex", [BX, NC], f16, isOutput=False)
    d_out = nc.declare_dram_parameter("out", [2, 128, NC], f16, isOutput=True)

    with tile.TileContext(nc) as tc:
        with (
            tc.tile_pool(name="const", bufs=1) as cpool,
            tc.tile_pool(name="ep0", bufs=3) as ep0,
            tc.tile_pool(name="epr", bufs=3) as epr,
            tc.tile_pool(name="op", bufs=1) as op,
            tc.tile_pool(name="psO", bufs=1, space="PSUM") as psO,
        ):
            EyR = cpool.tile([128, NI * NC], f16, tag="EyR")
            ExR = cpool.tile([128, NJ * NC], f16, tag="ExR")

            # compact tables + stride-0 broadcast replication, on the
            # scalar HW-DGE queue so they don't queue behind x. Order:
            # Ey row 0, all Ex chunks (row 0 consumes every j), rest of Ey.
            def ey_dma(i):
                nc.scalar.dma_start(
                    EyR[:, i * NC : (i + 1) * NC],
                    d_Ey[i * TY : (i + 1) * TY].unsqueeze(1).broadcast_to(
                        [TY, TX, NC]
                    ),
                )

            ey_dma(0)
            for j in range(NJ):
                nc.scalar.dma_start(
                    ExR[:, j * NC : (j + 1) * NC],
                    d_Ex[j * TX : (j + 1) * TX].unsqueeze(0).broadcast_to(
                        [TY, TX, NC]
                    ),
                )
            for i in range(1, NI):
                ey_dma(i)

            # x streams one 128-pixel tile per DMA on the sync HW-DGE
            # queue: in-order delivery at ~182ns/tile keeps ahead of the
            # 266ns/tile PE consumption; each LDWEIGHTS waits on its own
            # tile's sem only.
            xfull = cpool.tile([128, NT * B], f16, tag="xfull")
            for t in range(NT):
                nc.sync.dma_start(xfull[:, t * B : (t + 1) * B], d_xT[t])

            Op0 = psO.tile([128, 512], f32, tag="op0")
            Op1 = psO.tile([128, 512], f32, tag="op1")
            Jp = psO.tile([128, 512], f32, tag="junkp")

            # warm-up: waits only on the tiny Ey0 table DMA (~0.25us), so
            # the PE clock starts ramping well before real data lands
            for _ in range(WARMUP):
                nc.tensor.matmul(
                    Jp[:, 0:NC],
                    EyR[:, 0:128],
                    EyR[:, 0:NC],
                    start=True,
                    stop=True,
                    skip_group_check=True,
                )

            rows = {}

            def emit_row(i):
                er = epr.tile([128, NJ * NC], f16, tag="er")
                nc.vector.tensor_mul(
                    er[:].rearrange("p (j c) -> p j c", j=NJ),
                    EyR[:, i * NC : (i + 1) * NC].unsqueeze(1).broadcast_to(
                        [128, NJ, NC]
                    ),
                    ExR[:].rearrange("p (j c) -> p j c", j=NJ),
                )
                rows[i] = er

            # row 0 as per-tile multiplies so tile (0,j) only waits on Ex
            # chunk j (not the whole Ex table)
            tiles0 = [None] * NJ

            def emit_tile0(j):
                e = ep0.tile([128, NC], f16, tag="e0")
                nc.vector.tensor_mul(
                    e[:], EyR[:, 0:NC], ExR[:, j * NC : (j + 1) * NC]
                )
                tiles0[j] = e

            for j in range(3):
                emit_tile0(j)

            def emit_main(t, e):
                first, last = t == 0, t == NT - 1
                for bb, Opx in ((0, Op0), (1, Op1)):
                    nc.tensor.matmul(
                        Opx[:, 0:NC],
                        xfull[:, t * B + bb * 128 : t * B + (bb + 1) * 128],
                        e,
                        start=first,
                        stop=last,
                        skip_group_check=True,
                    )

            for t in range(NT):
                i, j = divmod(t, NJ)
                if i == 0 and j + 3 < NJ:
                    emit_tile0(j + 3)
                if j == 4 and i + 1 < NI:
                    emit_row(i + 1)
                e = tiles0[j][:] if i == 0 else rows[i][:, j * NC : (j + 1) * NC]
                emit_main(t, e)

            out_sb = op.tile([128, 2 * NC], f16, tag="out")
            nc.scalar.copy(out_sb[:, 0:NC], Op0[:, 0:NC])
            nc.vector.tensor_copy(out_sb[:, NC : 2 * NC], Op1[:, 0:NC])
            nc.sync.dma_start(d_out[0], out_sb[:, 0:NC])
            nc.sync.dma_start(d_out[1], out_sb[:, NC : 2 * NC])

    nc.compile()
    return nc


def _prepare(x, positions, sigmas, curve_weights, xs, ys):
    x = np.asarray(x, dtype=np.float32)
    px = np.asarray(positions, dtype=np.float64)[0, 0, :, 1]
    py = np.asarray(positions, dtype=np.float64)[0, 0, :, 0]
    sg = np.asarray(sigmas, dtype=np.float64)[0, 0]
    w = np.asarray(curve_weights, dtype=np.float64)[0, 0]
    xs = np.asarray(xs, dtype=np.float64)
    ys = np.asarray(ys, dtype=np.float64)

    # separability requires xs constant along rows, ys along cols
    assert np.allclose(xs, xs[0:1, :]) and np.allclose(ys, ys[:, 0:1])
    xs_ax = xs[0, :]
    ys_ax = ys[:, 0]

    s2 = 2.0 * sg * sg + EPS
    factor = 1.0 / (2.0 * np.pi * sg * sg + EPS)
    fw = factor * w
    # clip(curves) is identity when max|factor*w| <= CAP since exp(...) <= 1
    assert np.abs(fw).max() <= CAP, "clip binds; folded-scale scheme invalid"

    in_maps = []
    keep_idx = []
    for d in range(NDEV):
        iy, ix = d // GX, d % GX
        y0, x0 = iy * BY, ix * BX
        rows = ys_ax[y0 : y0 + BY]
        cols = xs_ax[x0 : x0 + BX]

        # 2D prune: closest-corner distance^2 / s2, cap at NC
        my = np.maximum(np.maximum(rows[0] - py, py - rows[-1]), 0.0)
        mx = np.maximum(np.maximum(cols[0] - px, px - cols[-1]), 0.0)
        score = (my * my + mx * mx) / s2
        idx = np.where(score <= T_PRUNE)[0]
        if len(idx) > NC:
            idx = idx[np.argsort(score[idx], kind="stable")[:NC]]
            idx.sort()
        nk = len(idx)
        keep_idx.append(idx)

        # compact 1-D exp tables over the block's rows/cols (padded cols 0)
        Ey = np.zeros((BY, NC), np.float16)
        Ex = np.zeros((BX, NC), np.float16)
        Ey[:, :nk] = np.exp(-((rows[:, None] - py[idx]) ** 2) / s2[idx])
        Ex[:, :nk] = np.exp(-((cols[:, None] - px[idx]) ** 2) / s2[idx])

        # x tile layout: xT[t=(i*NJ+j), l=(r*TX+wi), b] = x[b, y0+TY*i+r, x0+TX*j+wi]
        xb = x[:, y0 : y0 + BY, x0 : x0 + BX]
        xT = np.ascontiguousarray(
            xb.reshape(B, NI, TY, NJ, TX).transpose(1, 3, 2, 4, 0).reshape(NT, 128, B)
        ).astype(np.float16)

        in_maps.append({"xT": xT, "Ey": Ey, "Ex": Ex})
    return in_maps, keep_idx, fw


def _gather(results, keep_idx, fw):
    out = np.zeros((B, C), np.float32)
    for d in range(NDEV):
        idx = keep_idx[d]
        nk = len(idx)
        dev = np.asarray(results[d]["out"], np.float32).reshape(B, NC)
        out[:, idx] += dev[:, :nk] * (fw[idx] / NPIX).astype(np.float32)
    return out


def kernel(x, positions, sigmas, curve_weights, xs, ys):
    global last_results
    in_maps, keep_idx, fw = _prepare(x, positions, sigmas, curve_weights, xs, ys)
    nc = _build_program()
    trace = bool(os.environ.get("BLOB_TRACE"))
    last_results = run_bass_kernel_spmd(
        nc, in_maps, list(range(NDEV)), trace=trace
    )
    return _gather(last_results.results, keep_idx, fw)


# revision 8
# speedup vs baseline: 1.2386x; 1.2386x over previous
"""Trainium2 Bass kernel for the blob-layer problem.

Computes out[b, c] = sum_hw x[b, hw] * curves[hw, c] / (H*W) where
curves[hw, c] = clip(factor_c * exp(-((xs-px_c)^2 + (ys-py_c)^2)/s2_c) * w_c).

Strategy (8 NeuronCores, SPMD):
- The Gaussian is SEPARABLE: exp(-((x-px)^2+(y-py)^2)/s2) =
  Ex[w,c] * Ey[h,c]. Host computes the tiny 1-D tables; the device
  builds each tile's exp field with DVE multiplies of partition-
  replicated table tiles — no G matmul, no ACT Exp. The tensor engine
  does ONLY the main contraction: 2 fp16 matmuls per 128-pixel tile
  (one per 128-batch chunk) accumulating into PSUM.
- 2D core grid (4 y-bands x 2 x-halves), block = 56x112 pixels per
  core, tiled 7x7 as (8 rows x 16 cols) 128-pixel tiles. 2D pruning
  keeps columns with corner distance^2/s2 <= T=9, capped at NC=320
  (measured pruning-only rel err 1.7e-3 vs the 2e-2 gate).
- TRN2 ramps the PE clock 0.65 -> 1.2 -> 2.4 GHz with ~3us of
  continuous execution, so the schedule keeps the PE saturated:
  x streams per-tile on the sync HW-DGE queue (512B/partition
  descriptors, delivery 182ns/tile < 266ns/tile consumption),
  tables go compact over the scalar queue and are replicated by
  stride-0 broadcast DMAs, junk warm-up matmuls start the clock
  ramp before real data lands, and e is produced a full 7-tile row
  per DVE op (free-dim broadcast of Ey against the whole Ex row).
- factor*w/npix, the column gather/unpad, and the cross-core sum are
  applied on the host; the clip never binds when max|factor*w| <=
  CAP (exp <= 1), which is asserted.
"""
import os
import sys

sys.path.insert(0, "/opt/trn_rl_repo")

import numpy as np

import concourse.bass as bass
import concourse.bacc as bacc
import concourse.tile as tile
from concourse import mybir
from concourse.bass_utils import run_bass_kernel_spmd

H, W, B, C = 224, 224, 256, 1024
NDEV = 8
GY, GX = 4, 2             # core grid: 4 y-bands x 2 x-halves
BY, BX = H // GY, W // GX  # 56 x 112 block per core
TY, TX = 8, 16            # tile = 8 rows x 16 cols = 128 pixels
NI, NJ = BY // TY, BX // TX  # 7 x 7 tiles
NT = NI * NJ              # 49 tiles
NC = 320                  # kept/padded columns per core
EPS = 0.001
CAP = 2000.0
NPIX = float(H * W)
T_PRUNE = 9.0
WARMUP = 4                # junk matmuls that start the PE clock ramp

last_results = None       # BassKernelResults of the most recent run (for profiling)


def _build_program():
    nc = bacc.Bacc()
    f32 = mybir.dt.float32
    f16 = mybir.dt.float16

    d_xT = nc.declare_dram_parameter("xT", [NT, 128, B], f16, isOutput=False)
    d_EyT = nc.declare_dram_parameter("EyT", [TY, NI * NC], f16, isOutput=False)
    d_ExT = nc.declare_dram_parameter("ExT", [TX, NJ * NC], f16, isOutput=False)
    d_out = nc.declare_dram_parameter("out", [2, 128, NC], f16, isOutput=True)

    # Ex chunks 0..EX_SMALL-1 ship as small broadcast DMAs on the scalar
    # queue (fast first-tile latency); the rest ride two big broadcast
    # DMAs on the gpsimd software-DGE queue (one 550ns-class issue each).
    EX_SMALL = 3

    with tile.TileContext(nc) as tc:
        with (
            tc.tile_pool(name="const", bufs=1) as cpool,
            tc.tile_pool(name="ep0", bufs=3) as ep0,
            tc.tile_pool(name="epr", bufs=3) as epr,
            tc.tile_pool(name="op", bufs=1) as op,
            tc.tile_pool(name="psO", bufs=1, space="PSUM") as psO,
        ):
            EyR = cpool.tile([128, NI * NC], f16, tag="EyR")
            ExR = cpool.tile([128, NJ * NC], f16, tag="ExR")

            # scalar HW queue: the chunks the first few tiles need,
            # replicated from the compact transposed tables by stride-0
            # broadcast (source walk (r, wi, c) matches partition l=r*TX+wi)
            nc.scalar.dma_start(
                EyR[:, 0:NC],
                d_EyT[:, 0:NC].unsqueeze(1).broadcast_to([TY, TX, NC]),
            )
            for j in range(EX_SMALL):
                nc.scalar.dma_start(
                    ExR[:, j * NC : (j + 1) * NC],
                    d_ExT[:, j * NC : (j + 1) * NC]
                    .unsqueeze(0)
                    .broadcast_to([TY, TX, NC]),
                )
            # gpsimd SWDGE queue: everything else in two big DMAs
            nc.gpsimd.dma_start(
                EyR[:, NC : NI * NC],
                d_EyT[:, NC : NI * NC]
                .unsqueeze(1)
                .broadcast_to([TY, TX, (NI - 1) * NC]),
            )
            nc.gpsimd.dma_start(
                ExR[:, EX_SMALL * NC : NJ * NC],
                d_ExT[:, EX_SMALL * NC : NJ * NC]
                .unsqueeze(0)
                .broadcast_to([TY, TX, (NJ - EX_SMALL) * NC]),
            )

            # x streams in progressive groups on the sync HW queue:
            # small first groups start compute early, 7-tile groups
            # amortize the ~550ns per-DMA issue cost afterwards.
            xfull = cpool.tile([128, NT * B], f16, tag="xfull")
            t0 = 0
            for g in (2, 3, 4, 5, 7, 7, 7, 7, 7):
                t1 = min(t0 + g, NT)
                nc.sync.dma_start(
                    xfull[:, t0 * B : t1 * B].rearrange(
                        "p (t b) -> p t b", t=t1 - t0
                    ),
                    d_xT[t0:t1].rearrange("t p b -> p t b"),
                )
                t0 = t1

            Op0 = psO.tile([128, 512], f32, tag="op0")
            Op1 = psO.tile([128, 512], f32, tag="op1")
            Jp = psO.tile([128, 512], f32, tag="junkp")

            # warm-up: waits only on the tiny Ey0 table DMA (~0.25us), so
            # the PE clock starts ramping well before real data lands
            for _ in range(WARMUP):
                nc.tensor.matmul(
                    Jp[:, 0:NC],
                    EyR[:, 0:128],
                    EyR[:, 0:NC],
                    start=True,
                    stop=True,
                    skip_group_check=True,
                )

            rows = {}

            def emit_row(i):
                er = epr.tile([128, NJ * NC], f16, tag="er")
                nc.vector.tensor_mul(
                    er[:].rearrange("p (j c) -> p j c", j=NJ),
                    EyR[:, i * NC : (i + 1) * NC].unsqueeze(1).broadcast_to(
                        [128, NJ, NC]
                    ),
                    ExR[:].rearrange("p (j c) -> p j c", j=NJ),
                )
                rows[i] = er

            # row 0 as per-tile multiplies so tile (0,j) only waits on Ex
            # chunk j (not the whole Ex table)
            tiles0 = [None] * NJ

            def emit_tile0(j):
                e = ep0.tile([128, NC], f16, tag="e0")
                nc.vector.tensor_mul(
                    e[:], EyR[:, 0:NC], ExR[:, j * NC : (j + 1) * NC]
                )
                tiles0[j] = e

            for j in range(3):
                emit_tile0(j)

            def emit_main(t, e):
                first, last = t == 0, t == NT - 1
                for bb, Opx in ((0, Op0), (1, Op1)):
                    nc.tensor.matmul(
                        Opx[:, 0:NC],
                        xfull[:, t * B + bb * 128 : t * B + (bb + 1) * 128],
                        e,
                        start=first,
                        stop=last,
                        skip_group_check=True,
                    )

            for t in range(NT):
                i, j = divmod(t, NJ)
                if i == 0 and j + 3 < NJ:
                    emit_tile0(j + 3)
                if j == 4 and i + 1 < NI:
                    emit_row(i + 1)
                e = tiles0[j][:] if i == 0 else rows[i][:, j * NC : (j + 1) * NC]
                emit_main(t, e)

            out_sb = op.tile([128, 2 * NC], f16, tag="out")
            nc.scalar.copy(out_sb[:, 0:NC], Op0[:, 0:NC])
            nc.vector.tensor_copy(out_sb[:, NC : 2 * NC], Op1[:, 0:NC])
            nc.sync.dma_start(d_out[0], out_sb[:, 0:NC])
            nc.sync.dma_start(d_out[1], out_sb[:, NC : 2 * NC])

    nc.compile()
    return nc


def _prepare(x, positions, sigmas, curve_weights, xs, ys):
    x = np.asarray(x, dtype=np.float32)
    px = np.asarray(positions, dtype=np.float64)[0, 0, :, 1]
    py = np.asarray(positions, dtype=np.float64)[0, 0, :, 0]
    sg = np.asarray(sigmas, dtype=np.float64)[0, 0]
    w = np.asarray(curve_weights, dtype=np.float64)[0, 0]
    xs = np.asarray(xs, dtype=np.float64)
    ys = np.asarray(ys, dtype=np.float64)

    # separability requires xs constant along rows, ys along cols
    assert np.allclose(xs, xs[0:1, :]) and np.allclose(ys, ys[:, 0:1])
    xs_ax = xs[0, :]
    ys_ax = ys[:, 0]

    s2 = 2.0 * sg * sg + EPS
    factor = 1.0 / (2.0 * np.pi * sg * sg + EPS)
    fw = factor * w
    # clip(curves) is identity when max|factor*w| <= CAP since exp(...) <= 1
    assert np.abs(fw).max() <= CAP, "clip binds; folded-scale scheme invalid"

    in_maps = []
    keep_idx = []
    for d in range(NDEV):
        iy, ix = d // GX, d % GX
        y0, x0 = iy * BY, ix * BX
        rows = ys_ax[y0 : y0 + BY]
        cols = xs_ax[x0 : x0 + BX]

        # 2D prune: closest-corner distance^2 / s2, cap at NC
        my = np.maximum(np.maximum(rows[0] - py, py - rows[-1]), 0.0)
        mx = np.maximum(np.maximum(cols[0] - px, px - cols[-1]), 0.0)
        score = (my * my + mx * mx) / s2
        idx = np.where(score <= T_PRUNE)[0]
        if len(idx) > NC:
            idx = idx[np.argsort(score[idx], kind="stable")[:NC]]
            idx.sort()
        nk = len(idx)
        keep_idx.append(idx)

        # compact 1-D exp tables over the block's rows/cols (padded cols 0),
        # transposed so partition r/wi reads one contiguous DRAM run
        Ey = np.zeros((BY, NC), np.float16)
        Ex = np.zeros((BX, NC), np.float16)
        Ey[:, :nk] = np.exp(-((rows[:, None] - py[idx]) ** 2) / s2[idx])
        Ex[:, :nk] = np.exp(-((cols[:, None] - px[idx]) ** 2) / s2[idx])
        EyT = np.ascontiguousarray(
            Ey.reshape(NI, TY, NC).transpose(1, 0, 2).reshape(TY, NI * NC)
        )
        ExT = np.ascontiguousarray(
            Ex.reshape(NJ, TX, NC).transpose(1, 0, 2).reshape(TX, NJ * NC)
        )

        # x tile layout: xT[t=(i*NJ+j), l=(r*TX+wi), b] = x[b, y0+TY*i+r, x0+TX*j+wi]
        xb = x[:, y0 : y0 + BY, x0 : x0 + BX]
        xT = np.ascontiguousarray(
            xb.reshape(B, NI, TY, NJ, TX).transpose(1, 3, 2, 4, 0).reshape(NT, 128, B)
        ).astype(np.float16)

        in_maps.append({"xT": xT, "EyT": EyT, "ExT": ExT})
    return in_maps, keep_idx, fw


def _gather(results, keep_idx, fw):
    out = np.zeros((B, C), np.float32)
    for d in range(NDEV):
        idx = keep_idx[d]
        nk = len(idx)
        dev = np.asarray(results[d]["out"], np.float32).reshape(B, NC)
        out[:, idx] += dev[:, :nk] * (fw[idx] / NPIX).astype(np.float32)
    return out


def kernel(x, positions, sigmas, curve_weights, xs, ys):
    global last_results
    in_maps, keep_idx, fw = _prepare(x, positions, sigmas, curve_weights, xs, ys)
    nc = _build_program()
    trace = bool(os.environ.get("BLOB_TRACE"))
    last_results = run_bass_kernel_spmd(
        nc, in_maps, list(range(NDEV)), trace=trace
    )
    return _gather(last_results.results, keep_idx, fw)


# revision 15
# speedup vs baseline: 1.4014x; 1.1314x over previous
"""Trainium2 Bass kernel for the blob-layer problem.

Computes out[b, c] = sum_hw x[b, hw] * curves[hw, c] / (H*W) where
curves[hw, c] = clip(factor_c * exp(-((xs-px_c)^2 + (ys-py_c)^2)/s2_c) * w_c).

Strategy (8 NeuronCores, SPMD):
- The Gaussian is SEPARABLE: exp(-((x-px)^2+(y-py)^2)/s2) =
  Ex[w,c] * Ey[h,c]. Host computes the tiny 1-D tables; the device
  builds each tile's exp field with DVE multiplies of partition-
  replicated table tiles — no G matmul, no ACT Exp. The tensor engine
  does ONLY the main contraction: 2 fp16 matmuls per 128-pixel tile
  (one per 128-batch chunk) accumulating into PSUM.
- 2D core grid (4 y-bands x 2 x-halves), block = 56x112 pixels per
  core, tiled 7x7 as (8 rows x 16 cols) 128-pixel tiles. 2D pruning
  keeps columns with corner distance^2/s2 <= T=9, capped at NC=320
  (measured pruning-only rel err 1.7e-3 vs the 2e-2 gate).
- TRN2 ramps the PE clock 0.65 -> 1.2 -> 2.4 GHz with ~3us of
  continuous execution, so the schedule keeps the PE saturated:
  x streams per-tile on the sync HW-DGE queue (512B/partition
  descriptors, delivery 182ns/tile < 266ns/tile consumption),
  tables go compact over the scalar queue and are replicated by
  stride-0 broadcast DMAs, junk warm-up matmuls start the clock
  ramp before real data lands, and e is produced a full 7-tile row
  per DVE op (free-dim broadcast of Ey against the whole Ex row).
- factor*w/npix, the column gather/unpad, and the cross-core sum are
  applied on the host; the clip never binds when max|factor*w| <=
  CAP (exp <= 1), which is asserted.
"""
import os
import sys

sys.path.insert(0, "/opt/trn_rl_repo")

import numpy as np

import concourse.bass as bass
import concourse.bacc as bacc
import concourse.tile as tile
from concourse import mybir
from concourse.bass_utils import run_bass_kernel_spmd

H, W, B, C = 224, 224, 256, 1024
NDEV = 8
GY, GX = 4, 2             # core grid: 4 y-bands x 2 x-halves
BY, BX = H // GY, W // GX  # 56 x 112 block per core
TY, TX = 8, 16            # tile = 8 rows x 16 cols = 128 pixels
NI, NJ = BY // TY, BX // TX  # 7 x 7 tiles
NT = NI * NJ              # 49 tiles
NC = 320                  # kept/padded columns per core
EPS = 0.001
CAP = 2000.0
NPIX = float(H * W)
T_PRUNE = 9.0
WARMUP = 6                # junk matmuls that start the PE clock ramp

last_results = None       # BassKernelResults of the most recent run (for profiling)


def _build_program():
    nc = bacc.Bacc()
    f32 = mybir.dt.float32
    f16 = mybir.dt.float16

    d_xT = nc.declare_dram_parameter("xT", [NT, 128, B], f16, isOutput=False)
    d_EyR = nc.declare_dram_parameter("EyR", [128, NI * NC], f16, isOutput=False)
    d_ExR = nc.declare_dram_parameter("ExR", [128, NJ * NC], f16, isOutput=False)
    d_out = nc.declare_dram_parameter("out", [2, 128, NC], f16, isOutput=True)

    with tile.TileContext(nc) as tc:
        with (
            tc.tile_pool(name="const", bufs=1) as cpool,
            tc.tile_pool(name="ep0", bufs=3) as ep0,
            tc.tile_pool(name="epr", bufs=3) as epr,
            tc.tile_pool(name="op", bufs=1) as op,
            tc.tile_pool(name="psO", bufs=1, space="PSUM") as psO,
        ):
            EyR = cpool.tile([128, NI * NC], f16, tag="EyR")
            ExR = cpool.tile([128, NJ * NC], f16, tag="ExR")

            # host-replicated tables on the scalar HW queue, 3 contiguous
            # DMAs: Ey chunk 0 small (unblocks warm-up + row 0 fast), the
            # whole Ex table (row 0 needs every j chunk), rest of Ey
            # (first needed by row 1 at ~4.5us)
            nc.scalar.dma_start(EyR[:, 0:NC], d_EyR[:, 0:NC])
            nc.scalar.dma_start(ExR[:], d_ExR[:])
            nc.scalar.dma_start(EyR[:, NC : NI * NC], d_EyR[:, NC : NI * NC])

            # x streams in progressive groups on the sync HW queue:
            # small first groups start compute early, 7-tile groups
            # amortize the ~550ns per-DMA issue cost afterwards.
            xfull = cpool.tile([128, NT * B], f16, tag="xfull")
            t0 = 0
            for g in (2, 3, 4, 5, 7, 7, 7, 7, 7):
                t1 = min(t0 + g, NT)
                nc.sync.dma_start(
                    xfull[:, t0 * B : t1 * B].rearrange(
                        "p (t b) -> p t b", t=t1 - t0
                    ),
                    d_xT[t0:t1].rearrange("t p b -> p t b"),
                )
                t0 = t1

            Op0 = psO.tile([128, 512], f32, tag="op0")
            Op1 = psO.tile([128, 512], f32, tag="op1")
            Jp = psO.tile([128, 512], f32, tag="junkp")

            # warm-up: waits only on the small Ey0 table DMA, so the PE
            # clock starts ramping well before real data lands
            for _ in range(WARMUP):
                nc.tensor.matmul(
                    Jp[:, 0:NC],
                    EyR[:, 0:128],
                    EyR[:, 0:NC],
                    start=True,
                    stop=True,
                    skip_group_check=True,
                )

            rows = {}

            def emit_row(i):
                er = epr.tile([128, NJ * NC], f16, tag="er")
                nc.vector.tensor_mul(
                    er[:].rearrange("p (j c) -> p j c", j=NJ),
                    EyR[:, i * NC : (i + 1) * NC].unsqueeze(1).broadcast_to(
                        [128, NJ, NC]
                    ),
                    ExR[:].rearrange("p (j c) -> p j c", j=NJ),
                )
                rows[i] = er

            # row 0 as per-tile multiplies so tile (0,j) only waits on Ex
            # chunk j (not the whole Ex table)
            tiles0 = [None] * NJ

            def emit_tile0(j):
                e = ep0.tile([128, NC], f16, tag="e0")
                nc.vector.tensor_mul(
                    e[:], EyR[:, 0:NC], ExR[:, j * NC : (j + 1) * NC]
                )
                tiles0[j] = e

            for j in range(3):
                emit_tile0(j)

            def emit_main(t, e):
                first, last = t == 0, t == NT - 1
                for bb, Opx in ((0, Op0), (1, Op1)):
                    nc.tensor.matmul(
                        Opx[:, 0:NC],
                        xfull[:, t * B + bb * 128 : t * B + (bb + 1) * 128],
                        e,
                        start=first,
                        stop=last,
                        skip_group_check=True,
                    )

            for t in range(NT):
                i, j = divmod(t, NJ)
                if i == 0 and j + 3 < NJ:
                    emit_tile0(j + 3)
                if j == 4 and i + 1 < NI:
                    emit_row(i + 1)
                e = tiles0[j][:] if i == 0 else rows[i][:, j * NC : (j + 1) * NC]
                emit_main(t, e)

            # all-DVE tail: no activation op anywhere keeps the preamble's
            # ACT table load out of the critical path; one merged out DMA
            out_sb = op.tile([128, 2 * NC], f16, tag="out")
            nc.vector.tensor_copy(out_sb[:, 0:NC], Op0[:, 0:NC])
            nc.vector.tensor_copy(out_sb[:, NC : 2 * NC], Op1[:, 0:NC])
            nc.scalar.dma_start(
                d_out.rearrange("k p c -> p k c"),
                out_sb[:].rearrange("p (k c) -> p k c", k=2),
            )

    nc.compile()
    return nc


def _prepare(x, positions, sigmas, curve_weights, xs, ys):
    x = np.asarray(x, dtype=np.float32)
    px = np.asarray(positions, dtype=np.float64)[0, 0, :, 1]
    py = np.asarray(positions, dtype=np.float64)[0, 0, :, 0]
    sg = np.asarray(sigmas, dtype=np.float64)[0, 0]
    w = np.asarray(curve_weights, dtype=np.float64)[0, 0]
    xs = np.asarray(xs, dtype=np.float64)
    ys = np.asarray(ys, dtype=np.float64)

    # separability requires xs constant along rows, ys along cols
    assert np.allclose(xs, xs[0:1, :]) and np.allclose(ys, ys[:, 0:1])
    xs_ax = xs[0, :]
    ys_ax = ys[:, 0]

    s2 = 2.0 * sg * sg + EPS
    factor = 1.0 / (2.0 * np.pi * sg * sg + EPS)
    fw = factor * w
    # clip(curves) is identity when max|factor*w| <= CAP since exp(...) <= 1
    assert np.abs(fw).max() <= CAP, "clip binds; folded-scale scheme invalid"

    in_maps = []
    keep_idx = []
    for d in range(NDEV):
        iy, ix = d // GX, d % GX
        y0, x0 = iy * BY, ix * BX
        rows = ys_ax[y0 : y0 + BY]
        cols = xs_ax[x0 : x0 + BX]

        # 2D prune: closest-corner distance^2 / s2, cap at NC
        my = np.maximum(np.maximum(rows[0] - py, py - rows[-1]), 0.0)
        mx = np.maximum(np.maximum(cols[0] - px, px - cols[-1]), 0.0)
        score = (my * my + mx * mx) / s2
        idx = np.where(score <= T_PRUNE)[0]
        if len(idx) > NC:
            idx = idx[np.argsort(score[idx], kind="stable")[:NC]]
            idx.sort()
        nk = len(idx)
        keep_idx.append(idx)

        # 1-D exp tables over the block's rows/cols (padded cols 0),
        # partition-replicated on host: EyR[l=(r*TX+wi), i*NC+c] =
        # Ey[TY*i+r, c]; ExR[l, j*NC+c] = Ex[TX*j+wi, c]
        Ey = np.zeros((BY, NC), np.float16)
        Ex = np.zeros((BX, NC), np.float16)
        Ey[:, :nk] = np.exp(-((rows[:, None] - py[idx]) ** 2) / s2[idx])
        Ex[:, :nk] = np.exp(-((cols[:, None] - px[idx]) ** 2) / s2[idx])
        EyR = np.ascontiguousarray(
            np.broadcast_to(
                Ey.reshape(NI, TY, 1, NC), (NI, TY, TX, NC)
            ).transpose(1, 2, 0, 3).reshape(128, NI * NC)
        )
        ExR = np.ascontiguousarray(
            np.broadcast_to(
                Ex.reshape(1, NJ, TX, NC), (TY, NJ, TX, NC)
            ).transpose(0, 2, 1, 3).reshape(128, NJ * NC)
        )

        # x tile layout: xT[t=(i*NJ+j), l=(r*TX+wi), b] = x[b, y0+TY*i+r, x0+TX*j+wi]
        xb = x[:, y0 : y0 + BY, x0 : x0 + BX]
        xT = np.ascontiguousarray(
            xb.reshape(B, NI, TY, NJ, TX).transpose(1, 3, 2, 4, 0).reshape(NT, 128, B)
        ).astype(np.float16)

        in_maps.append({"xT": xT, "EyR": EyR, "ExR": ExR})
    return in_maps, keep_idx, fw


def _gather(results, keep_idx, fw):
    out = np.zeros((B, C), np.float32)
    for d in range(NDEV):
        idx = keep_idx[d]
        nk = len(idx)
        dev = np.asarray(results[d]["out"], np.float32).reshape(B, NC)
        out[:, idx] += dev[:, :nk] * (fw[idx] / NPIX).astype(np.float32)
    return out


def kernel(x, positions, sigmas, curve_weights, xs, ys):
    global last_results
    in_maps, keep_idx, fw = _prepare(x, positions, sigmas, curve_weights, xs, ys)
    nc = _build_program()
    trace = bool(os.environ.get("BLOB_TRACE"))
    last_results = run_bass_kernel_spmd(
        nc, in_maps, list(range(NDEV)), trace=trace
    )
    return _gather(last_results.results, keep_idx, fw)
